# revision 1
# baseline (speedup 1.0000x reference)
"""Multi-head causal attention (B=2, S=2048, D=1024, H=16) on 8 TRN2 NeuronCores.

v2: Megatron head-parallel attention + SEQUENCE-PARALLEL output projection.
Core c owns heads {2c, 2c+1}:
  - W_q/W_k/W_v column slices [:, 128c:128(c+1)]  (2 heads x 64 dims)
  - attention for those heads over the full sequence (causal)
  - normalized context slices are exchanged with AllToAll (bf16, 5 chunks,
    ~1/8 the traffic of an AllGather) so each core ends up with the FULL
    context (all 1024 dims) for 512 of the 4096 sequence rows
  - each core computes out[myrows, :] = ctx_full[myrows, :] @ W_o (full W_o)
  - host concatenates the 8 row slices (pure gather, no arithmetic)

Compute dtype: bf16 operands, fp32 PSUM accumulation. Scores are computed
transposed (S^T[k,q] = K Q^T) so the P^T tiles feed the A@V matmul directly;
softmax denominators come from an extra all-ones column appended to V.

x is cast fp32->bf16 *during* the DMA (SWDGE cast), transposed on the
TensorEngine, and the per-512-row-chunk QKV + x-transpose work is streamed
through a filler queue that attention drains between its score/AV matmuls --
the TensorEngine never idles behind the (slower) ScalarEngine softmax exp,
keeping the PE HAM clock at full rate.
"""

from collections import deque

import numpy as np

import concourse.bass as bass
import concourse.mybir as mybir
from concourse import bacc, tile
from concourse.masks import make_identity
from concourse.bass_utils import run_bass_kernel_spmd

N_CORES = 8
B, S, D = 2, 2048, 1024
H, DH = 16, 64
BS = B * S  # 4096
HPC = H // N_CORES  # heads per core = 2
DHC = HPC * DH  # 128 context dims per core
SCALE = 1.0 / 32.0  # 1/sqrt(D)
FP32 = mybir.dt.float32
BF16 = mybir.dt.bfloat16
Exp = mybir.ActivationFunctionType.Exp

NQ = 4  # q macro tiles of 512 per batch element
QM = S // NQ  # 512
NKT = S // 128  # 16 k-tiles of 128 per batch element

# per-core output rows: [m0:128 | m1:128 | m2:128 | m3a:64 | m3b:64]
OUT_ROWS = 512

_nc_cache = {}


def _build():
    nc = bacc.Bacc(
        "TRN2", target_bir_lowering=False, debug=False, num_devices=N_CORES
    )

    x_d = nc.dram_tensor("x", [BS, D], FP32, kind="ExternalInput").ap()
    wq_d = nc.dram_tensor("wq", [D, DHC], FP32, kind="ExternalInput").ap()
    wk_d = nc.dram_tensor("wk", [D, DHC], FP32, kind="ExternalInput").ap()
    wv_d = nc.dram_tensor("wv", [D, DHC], FP32, kind="ExternalInput").ap()
    wo_d = nc.dram_tensor("wo", [D, D], FP32, kind="ExternalInput").ap()
    tri_d = nc.dram_tensor("tri", [128, 128], FP32, kind="ExternalInput").ap()
    out_d = nc.dram_tensor("out", [OUT_ROWS, D], FP32, kind="ExternalOutput").ap()
    import os as _os
    DBG = _os.environ.get("KERNEL2_DEBUG", "0") == "1"
    if DBG:
        dbg_xt = nc.dram_tensor("dbg_xt", [128, 8 * 512], FP32, kind="ExternalOutput").ap()
        dbg_qt = nc.dram_tensor("dbg_qt", [128, 512], FP32, kind="ExternalOutput").ap()
        dbg_kt = nc.dram_tensor("dbg_kt", [128, 512], FP32, kind="ExternalOutput").ap()
        dbg_v = nc.dram_tensor("dbg_v", [128, 520], FP32, kind="ExternalOutput").ap()
        dbg_ctxn = nc.dram_tensor("dbg_ctxn", [128, 512], FP32, kind="ExternalOutput").ap()
        dbg_cf = nc.dram_tensor("dbg_cf", [128, 8 * 128], FP32, kind="ExternalOutput").ap()
        dbg_im = nc.dram_tensor("dbg_im", [1024, 128], FP32, kind="ExternalOutput").ap()
        dbg_om = nc.dram_tensor("dbg_om", [1024, 128], FP32, kind="ExternalOutput").ap()

    with tile.TileContext(nc) as tc:
        with (
            tc.tile_pool(name="dram", bufs=1, space="DRAM") as dram,
            tc.tile_pool(name="pers", bufs=1) as pers,
            tc.tile_pool(name="xfp", bufs=2) as xfp,
            tc.tile_pool(name="xbp", bufs=2) as xbp,
            tc.tile_pool(name="ptp", bufs=6) as ptp,
            tc.tile_pool(name="nw", bufs=3) as nw,
            tc.tile_pool(name="cfp", bufs=2) as cfp,
            tc.tile_pool(name="ps_s", bufs=2, space="PSUM") as ps_s,
            tc.tile_pool(name="ps_c", bufs=2, space="PSUM") as ps_c,
            tc.tile_pool(name="ps_m", bufs=2, space="PSUM") as ps_m,
        ):
            # ---- persistent SBUF ----
            xt_sb = pers.tile([128, 8, BS], BF16, name="xt_sb")
            qt_sb = [pers.tile([128, S], BF16, name=f"qt{b}") for b in range(B)]
            kt_sb = [pers.tile([128, S], BF16, name=f"kt{b}") for b in range(B)]
            # V tiles: per k-tile layout [h0 64 | ones | h1 64 | ones] (130 cols)
            v_sb = [pers.tile([128, NKT * 130], BF16, name=f"v{b}") for b in range(B)]
            wq_sb = pers.tile([128, 8, DHC], BF16, name="wq_sb")
            wk_sb = pers.tile([128, 8, DHC], BF16, name="wk_sb")
            wv_sb = pers.tile([128, 8, DHC], BF16, name="wv_sb")
            wo_sb = pers.tile([128, 8, D], BF16, name="wo_sb")
            tri_sb = pers.tile([128, 128], BF16, name="tri_sb")
            ones_sb = pers.tile([1, 64], BF16, name="ones_sb")
            id_b = pers.tile([128, 128], BF16, name="id_b")

            # ---- A2A staging: input (per-rank shards) and Shared output ----
            # k=0..2: (both b, m=k), 8 shards x [128, 128]
            # k=3: (b=0, m=3), k=4: (b=1, m=3): 8 shards x [128, 64]
            im_c = [
                dram.tile([N_CORES * DHC, 128], BF16, name=f"im{k}")
                for k in range(3)
            ] + [
                dram.tile([N_CORES * DHC, 64], BF16, name=f"im3{s}") for s in "ab"
            ]
            om_c = [
                dram.tile([N_CORES * DHC, 128], BF16, name=f"om{k}")
                for k in range(3)
            ] + [
                dram.tile([N_CORES * DHC, 64], BF16, name=f"om3{s}") for s in "ab"
            ]

            # warm-up collectives: the first two collectives of a NEFF pay a
            # large one-time setup (~40us + ~20us measured). Fire two tiny
            # AllToAlls at t=0 (no input deps -> no ring blocking) so the real
            # exchanges hit steady-state (~6us) cost.
            # id_b first: every x^T transpose depends on it, and gpsimd-ring
            # position IS execution order.
            make_identity(nc, id_b[:])
            nc.vector.memset(ones_sb[:], 1.0)
            nc.vector.memset(v_sb[0][:], 1.0)
            nc.vector.memset(v_sb[1][:], 1.0)
            dmy_i = dram.tile([8, 64], BF16, name="dmy_i")
            dmy_o = [dram.tile([8, 64], BF16, name=f"dmy_o{t}") for t in range(1)]
            for t_ in range(1):
                nc.gpsimd.collective_compute(
                    "AllToAll",
                    mybir.AluOpType.bypass,
                    replica_groups=[list(range(N_CORES))],
                    ins=[dmy_i[:]],
                    outs=[dmy_o[t_][:]],
                )

            # ---- gpsimd (SWDGE) ring: x cast-DMAs + weights; collectives later.
            # First 5 chunks upfront (pool bufs=5); chunks 5..7 are emitted later
            # (after the readers of the buffer they reuse), keeping all x DMAs
            # ahead of every AllToAll trigger-wait on this ring.
            x_bufs = {}
            x_fbufs = {}

            def x_dma(b, j):
                g = b * 4 + j
                rows = slice(g * 512, (g + 1) * 512)
                x_f = xfp.tile([128, 4, D], FP32, name="x_f", tag="xf")
                nc.sync.dma_start(
                    x_f[:], x_d[rows, :].rearrange("(c p) d -> p c d", p=128)
                )
                x_fbufs[(b, j)] = x_f

            x_dma(0, 0)
            for w_d, w_sb in ((wq_d, wq_sb), (wk_d, wk_sb), (wv_d, wv_sb)):
                nc.gpsimd.dma_start(
                    w_sb[:], w_d.rearrange("(c p) n -> p c n", p=128)
                )
            nc.gpsimd.dma_start(tri_sb[:], tri_d[:])
            x_dma(1, 0)
            nc.gpsimd.dma_start(
                wo_sb[:], wo_d.rearrange("(c p) n -> p c n", p=128)
            )

            # ---- filler queue ----
            Q = deque()

            def push(fn, tag=None):
                Q.append((tag, fn))

            def fill(n):
                for _ in range(min(n, len(Q))):
                    Q.popleft()[1]()

            def drain_until(tag):
                while Q:
                    t, fn = Q.popleft()
                    fn()
                    if t == tag:
                        break

            def drain_all():
                while Q:
                    Q.popleft()[1]()

            # toggle for eviction engine (spread PSUM-evict copies over
            # Vector and Scalar so neither becomes the bottleneck)
            _tgl = [0]

            def evict(dst, src):
                _tgl[0] ^= 1
                if _tgl[0]:
                    nc.vector.tensor_copy(dst, src)
                else:
                    nc.scalar.copy(dst, src)

            # ---- per-chunk prep: x^T (PE transpose) + QKV projections ----
            _proj_ps = {}
            _vt_sb = {}

            def push_prep(b, j):
                g = b * 4 + j
                cols = slice(b * S + j * QM, b * S + (j + 1) * QM)

                def cast():
                    x_f = x_fbufs.pop((b, j))
                    x_b = xbp.tile([128, 4, D], BF16, name="x_b", tag="xb")
                    nc.vector.tensor_copy(x_b[:], x_f[:])
                    x_bufs[(b, j)] = x_b

                push(cast)

                def tp(dt0):
                    def f():
                        x_b = x_bufs[(b, j)]
                        pst = ps_m.tile([128, 1024], BF16, name="pst", tag="m")
                        for dd in range(2):
                            dt = dt0 + dd
                            for c in range(4):
                                nc.tensor.transpose(
                                    pst[
                                        :,
                                        dd * 512 + c * 128 : dd * 512 + (c + 1) * 128,
                                    ],
                                    x_b[:, c, dt * 128 : (dt + 1) * 128],
                                    id_b[:],
                                )
                        nc.vector.tensor_copy(
                            xt_sb[:, dt0 : dt0 + 2, g * 512 : (g + 1) * 512],
                            pst[:].rearrange("p (d c) -> p d c", d=2),
                        )

                    return f

                for dt0 in (0, 2, 4, 6):
                    push(tp(dt0))

                def proj(w_sb, t_sb, half):
                    def f():
                        key = (b, j, id(w_sb))
                        if half == 0:
                            ps = ps_m.tile([128, QM], FP32, name="ps_p", tag="m")
                            _proj_ps[key] = ps
                        else:
                            ps = _proj_ps.pop(key)
                        for dt in range(4 * half, 4 * half + 4):
                            nc.tensor.matmul(
                                ps[:],
                                w_sb[:, dt, :],
                                xt_sb[:, dt, cols],
                                start=(dt == 0),
                                stop=(dt == 7),
                            )
                        if half == 1:
                            if t_sb is not None:
                                evict(t_sb[:, j * QM : (j + 1) * QM], ps[:])
                            else:
                                vt = nw.tile(
                                    [128, QM], BF16, name="vt", tag="vt", bufs=2
                                )
                                nc.vector.tensor_copy(vt[:], ps[:])
                                _vt_sb[(b, j)] = vt

                    return f

                push(proj(wq_sb, qt_sb[b], 0))
                push(proj(wq_sb, qt_sb[b], 1))
                push(proj(wk_sb, kt_sb[b], 0))
                push(proj(wk_sb, kt_sb[b], 1))
                push(proj(wv_sb, None, 0))
                push(proj(wv_sb, None, 1))

                def vtp():
                    vt = _vt_sb.pop((b, j))
                    pst = ps_m.tile([128, 1024], BF16, name="pst", tag="m")
                    for st in range(4):
                        nc.tensor.transpose(
                            pst[:, st * 128 : (st + 1) * 128],
                            vt[:, st * 128 : (st + 1) * 128],
                            id_b[:],
                        )
                    dst = v_sb[b][:, j * 520 : (j + 1) * 520].rearrange(
                        "p (t g c) -> p t g c", t=4, g=2
                    )[:, :, :, 0:64]
                    vsrc = pst[:, 0:512].rearrange("p (t g c) -> p t g c", t=4, g=2)
                    nc.vector.tensor_copy(dst, vsrc)

                push(vtp, tag=f"prep{b}{j}")

            # ---- attention for (b, m): scores + softmax + A@V; returns tail ----
            def attention(b, m):
                qcols = slice(m * QM, (m + 1) * QM)
                ctx_ps = [
                    ps_c.tile([65, QM], FP32, name=f"ctx_ps{h}", tag="c")
                    for h in range(HPC)
                ]
                n_kt = 4 * m + 4

                def score_mm(kt):
                    s_ps = ps_s.tile([128, 2 * QM], FP32, name="s_ps", tag="s")
                    for h in range(HPC):
                        nc.tensor.matmul(
                            s_ps[:, h * QM : (h + 1) * QM],
                            kt_sb[b][h * 64 : (h + 1) * 64, kt * 128 : (kt + 1) * 128],
                            qt_sb[b][h * 64 : (h + 1) * 64, qcols],
                            start=True,
                            stop=True,
                            tile_position=(h * 64, 0),
                        )
                    return s_ps

                s_cur = score_mm(0)
                for kt in range(n_kt):
                    s_nxt = score_mm(kt + 1) if kt + 1 < n_kt else None
                    s_ps = s_cur
                    j = kt - 4 * m  # diagonal block index if >= 0
                    qs = max(0, 128 * j)
                    pt = ptp.tile([128, 2 * QM], BF16, name="pt")
                    if j < 0:
                        nc.scalar.activation(pt[:], s_ps[:], Exp, scale=SCALE)
                    else:
                        for h in range(HPC):
                            nc.scalar.activation(
                                pt[:, h * QM + qs : (h + 1) * QM],
                                s_ps[:, h * QM + qs : (h + 1) * QM],
                                Exp,
                                scale=SCALE,
                            )
                            nc.vector.tensor_mul(
                                pt[:, h * QM + qs : h * QM + qs + 128],
                                pt[:, h * QM + qs : h * QM + qs + 128],
                                tri_sb[:],
                            )
                    fill(2)
                    for h in range(HPC):
                        nc.tensor.matmul(
                            ctx_ps[h][:, qs:QM],
                            v_sb[b][:, kt * 130 + h * 65 : kt * 130 + (h + 1) * 65],
                            pt[:, h * QM + qs : (h + 1) * QM],
                            start=(kt == 0),
                            stop=(kt == n_kt - 1),
                        )
                    s_cur = s_nxt
                # evict accumulators to SBUF (frees PSUM) + reciprocals now;
                # the PE-side normalize tail is deferred.
                # evict + reciprocal on the SCALAR engine: its queue position is
                # right after this attention's own exps, so the tail chain that
                # gates the AllToAll trigger is not stuck behind the (deep)
                # vector-engine backlog.
                ctxa_l, recip_l = [], []
                for h in range(HPC):
                    ctxa = nw.tile([65, QM], FP32, name="ctxa", tag="ctxa", bufs=4)
                    nc.scalar.copy(ctxa[:], ctx_ps[h][:])
                    recip = nw.tile([1, QM], BF16, name="recip", tag="recip", bufs=4)
                    with nc.allow_low_precision(reason="softmax denom to bf16"):
                        nc.vector.reciprocal(recip[:], ctxa[64:65, :])
                    ctxa_l.append(ctxa)
                    recip_l.append(recip)

                def tail():
                    bc_ps = ps_m.tile([128, QM], FP32, name="bc_ps", tag="m")
                    for h in range(HPC):
                        nc.tensor.matmul(
                            bc_ps[64 * h : 64 * h + 64, :],
                            ones_sb[:],
                            recip_l[h][:],
                            start=True,
                            stop=True,
                        )
                    ctxn = nw.tile([128, QM], BF16, name="ctxn", tag="ctxn")
                    for h in range(HPC):
                        nc.vector.tensor_mul(
                            ctxn[64 * h : 64 * h + 64, :],
                            ctxa_l[h][0:64, :],
                            bc_ps[64 * h : 64 * h + 64, :],
                        )
                    if m < 3:
                        dst = im_c[m].rearrange("(s r) c -> r s c", r=128)[
                            :, 4 * b : 4 * b + 4, :
                        ]
                        src = ctxn[:].rearrange("r (p c) -> r p c", p=4)
                    else:
                        dst = im_c[3 + b].rearrange("(s r) c -> r s c", r=128)
                        src = ctxn[:].rearrange("r (s c) -> r s c", s=8)
                    nc.sync.dma_start(dst, src)
                    if DBG and b == 0 and m == 0:
                        nc.gpsimd.dma_start(dbg_ctxn[:, :], ctxn[:])

                return tail

            def a2a(k):
                nc.gpsimd.collective_compute(
                    "AllToAll",
                    mybir.AluOpType.bypass,
                    replica_groups=[list(range(N_CORES))],
                    ins=[im_c[k][:]],
                    outs=[om_c[k][:]],
                )

            # ---- output projection for A2A chunk k ----
            _cf = {}
            _of = {}

            def push_op(k):
                C = 128 if k < 3 else 64
                off = 128 * k if k < 3 else 384 + 64 * (k - 3)

                def cfdma():
                    cf = cfp.tile([128, 8, C], BF16, name="cf", tag="cf")
                    nc.gpsimd.dma_start(
                        cf[:], om_c[k].rearrange("(t p) c -> p t c", p=128)
                    )
                    _cf[k] = cf
                    if DBG and k == 0:
                        nc.gpsimd.dma_start(
                            dbg_cf.rearrange("p (t c) -> p t c", t=8), cf[:]
                        )

                def mm(half):
                    def f():
                        cf = _cf[k]
                        if half == 0:
                            of = nw.tile([C, D], FP32, name="of", tag="of", bufs=2)
                            _of[k] = of
                        ps = ps_m.tile([C, 512], FP32, name="ps_o", tag="m")
                        for dt in range(8):
                            nc.tensor.matmul(
                                ps[:],
                                cf[:, dt, :],
                                wo_sb[:, dt, half * 512 : (half + 1) * 512],
                                start=(dt == 0),
                                stop=(dt == 7),
                            )
                        evict(_of[k][:, half * 512 : (half + 1) * 512], ps[:])
                        if half == 1:
                            of = _of.pop(k)
                            nc.gpsimd.dma_start(out_d[off : off + C, :], of[:])

                    return f

                push(cfdma)
                push(mm(0))
                push(mm(1))

            # ---- main pipeline ----
            # chunk order: (0,0),(1,0),(0,1),(1,1),(0,2),(1,2),(0,3),(1,3)
            chunks = [(b, m) for m in range(NQ) for b in range(B)]
            push_prep(0, 0)
            drain_all()
            x_dma(0, 1)  # cast(0,0) emitted -> its x_f buffer is reusable
            # outproj thunks are spliced well after their AllToAll fires so the
            # strict-FIFO PE stream never blocks on a cf load.
            op_defer = {5: [0], 6: [1], 7: [2, 3]}
            for i, (b, m) in enumerate(chunks):
                if i + 1 < len(chunks):
                    nb, nm = chunks[i + 1]
                    push_prep(nb, nm)
                for k_ in op_defer.get(i, ()):
                    push_op(k_)
                t = attention(b, m)
                fill(2)
                t()
                if b == 1 and m < 3:
                    a2a(m)
                    if DBG and m == 0:
                        nc.gpsimd.dma_start(dbg_im[:, :], im_c[0][:])
                        nc.gpsimd.dma_start(dbg_om[:, :], om_c[0][:])
                if b == 0 and m == 3:
                    a2a(3)
                if i + 1 < len(chunks):
                    drain_until(f"prep{nb}{nm}")
                if i + 3 < len(chunks):
                    x_dma(*chunks[i + 3])  # cast(i+1) emitted -> buffer reusable
            a2a(4)
            drain_all()
            push_op(4)
            drain_all()
            if DBG:
                nc.gpsimd.dma_start(
                    dbg_xt.rearrange("p (t c) -> p t c", t=8), xt_sb[:, :, 0:512]
                )
                nc.gpsimd.dma_start(dbg_qt[:, :], qt_sb[0][:, 0:512])
                nc.gpsimd.dma_start(dbg_kt[:, :], kt_sb[0][:, 0:512])
                nc.gpsimd.dma_start(dbg_v[:, :], v_sb[0][:, 0:520])

    nc.compile()
    return nc


def _build_nc():
    if "nc" not in _nc_cache:
        _nc_cache["nc"] = _build()
    return _nc_cache["nc"]


def kernel(x, W_q, W_k, W_v, W_o):
    x = np.ascontiguousarray(np.asarray(x, dtype=np.float32)).reshape(BS, D)
    # keep-mask for the diagonal 128x128 block of S^T[k, q]: keep k <= q
    tri = np.triu(np.ones((128, 128), dtype=np.float32))
    wo_full = np.ascontiguousarray(np.asarray(W_o, np.float32))
    in_maps = []
    for c in range(N_CORES):
        sl = slice(c * DHC, (c + 1) * DHC)
        in_maps.append(
            {
                "x": x,
                "wq": np.ascontiguousarray(np.asarray(W_q, np.float32)[:, sl]),
                "wk": np.ascontiguousarray(np.asarray(W_k, np.float32)[:, sl]),
                "wv": np.ascontiguousarray(np.asarray(W_v, np.float32)[:, sl]),
                "wo": wo_full,
                "tri": tri,
            }
        )
    nc = _build_nc()
    res = run_bass_kernel_spmd(nc, in_maps, core_ids=list(range(N_CORES)))
    out = np.empty((B, S, D), dtype=np.float32)
    for c in range(N_CORES):
        oc = res.results[c]["out"]  # [512, 1024]
        bb, p = c // 4, c % 4
        for m in range(3):
            out[bb, m * QM + p * 128 : m * QM + (p + 1) * 128, :] = oc[
                m * 128 : (m + 1) * 128
            ]
        out[0, 3 * QM + 64 * c : 3 * QM + 64 * c + 64, :] = oc[384:448]
        out[1, 3 * QM + 64 * c : 3 * QM + 64 * c + 64, :] = oc[448:512]
    return out



# revision 9
# speedup vs baseline: 1.0955x; 1.0955x over previous
"""Multi-head causal attention (B=2, S=2048, D=1024, H=16) on 8 TRN2 NeuronCores.

v2: Megatron head-parallel attention + SEQUENCE-PARALLEL output projection.
Core c owns heads {2c, 2c+1}:
  - W_q/W_k/W_v column slices [:, 128c:128(c+1)]  (2 heads x 64 dims)
  - attention for those heads over the full sequence (causal)
  - normalized context slices are exchanged with AllToAll (bf16, 5 chunks,
    ~1/8 the traffic of an AllGather) so each core ends up with the FULL
    context (all 1024 dims) for 512 of the 4096 sequence rows
  - each core computes out[myrows, :] = ctx_full[myrows, :] @ W_o (full W_o)
  - host concatenates the 8 row slices (pure gather, no arithmetic)

Compute dtype: bf16 operands, fp32 PSUM accumulation. Scores are computed
transposed (S^T[k,q] = K Q^T) so the P^T tiles feed the A@V matmul directly;
softmax denominators come from an extra all-ones column appended to V.

x is cast fp32->bf16 *during* the DMA (SWDGE cast), transposed on the
TensorEngine, and the per-512-row-chunk QKV + x-transpose work is streamed
through a filler queue that attention drains between its score/AV matmuls --
the TensorEngine never idles behind the (slower) ScalarEngine softmax exp,
keeping the PE HAM clock at full rate.
"""

from collections import deque

import numpy as np

import concourse.bass as bass
import concourse.mybir as mybir
from concourse import bacc, tile
from concourse.masks import make_identity
from concourse.bass_utils import run_bass_kernel_spmd

N_CORES = 8
B, S, D = 2, 2048, 1024
H, DH = 16, 64
BS = B * S  # 4096
HPC = H // N_CORES  # heads per core = 2
DHC = HPC * DH  # 128 context dims per core
SCALE = 1.0 / 32.0  # 1/sqrt(D)
FP32 = mybir.dt.float32
BF16 = mybir.dt.bfloat16
Exp = mybir.ActivationFunctionType.Exp

NQ = 4  # q macro tiles of 512 per batch element
QM = S // NQ  # 512
NKT = S // 128  # 16 k-tiles of 128 per batch element

# per-core output rows: [m0:128 | m1:128 | m2:128 | m3a:64 | m3b:64]
OUT_ROWS = 512

_nc_cache = {}


def _build():
    nc = bacc.Bacc(
        "TRN2", target_bir_lowering=False, debug=False, num_devices=N_CORES
    )

    x_d = nc.dram_tensor("x", [BS, D], FP32, kind="ExternalInput").ap()
    wq_d = nc.dram_tensor("wq", [D, DHC], FP32, kind="ExternalInput").ap()
    wk_d = nc.dram_tensor("wk", [D, DHC], FP32, kind="ExternalInput").ap()
    wv_d = nc.dram_tensor("wv", [D, DHC], FP32, kind="ExternalInput").ap()
    wo_d = nc.dram_tensor("wo", [D, D], FP32, kind="ExternalInput").ap()
    tri_d = nc.dram_tensor("tri", [128, 128], FP32, kind="ExternalInput").ap()
    out_d = nc.dram_tensor("out", [OUT_ROWS, D], FP32, kind="ExternalOutput").ap()
    import os as _os
    DBG = _os.environ.get("KERNEL2_DEBUG", "0") == "1"
    if DBG:
        dbg_xt = nc.dram_tensor("dbg_xt", [128, 8 * 512], FP32, kind="ExternalOutput").ap()
        dbg_qt = nc.dram_tensor("dbg_qt", [128, 512], FP32, kind="ExternalOutput").ap()
        dbg_kt = nc.dram_tensor("dbg_kt", [128, 512], FP32, kind="ExternalOutput").ap()
        dbg_v = nc.dram_tensor("dbg_v", [128, 520], FP32, kind="ExternalOutput").ap()
        dbg_ctxn = nc.dram_tensor("dbg_ctxn", [128, 512], FP32, kind="ExternalOutput").ap()
        dbg_cf = nc.dram_tensor("dbg_cf", [128, 8 * 128], FP32, kind="ExternalOutput").ap()
        dbg_im = nc.dram_tensor("dbg_im", [1024, 128], FP32, kind="ExternalOutput").ap()
        dbg_om = nc.dram_tensor("dbg_om", [1024, 128], FP32, kind="ExternalOutput").ap()

    with tile.TileContext(nc) as tc:
        with (
            tc.tile_pool(name="dram", bufs=1, space="DRAM") as dram,
            tc.tile_pool(name="pers", bufs=1) as pers,
            tc.tile_pool(name="xfp", bufs=2) as xfp,
            tc.tile_pool(name="xbp", bufs=2) as xbp,
            tc.tile_pool(name="ptp", bufs=6) as ptp,
            tc.tile_pool(name="nw", bufs=3) as nw,
            tc.tile_pool(name="cfp", bufs=2) as cfp,
            tc.tile_pool(name="ps_s", bufs=2, space="PSUM") as ps_s,
            tc.tile_pool(name="ps_c", bufs=2, space="PSUM") as ps_c,
            tc.tile_pool(name="ps_m", bufs=2, space="PSUM") as ps_m,
        ):
            # ---- persistent SBUF ----
            xt_sb = pers.tile([128, 8, BS], BF16, name="xt_sb")
            qt_sb = [pers.tile([128, S], BF16, name=f"qt{b}") for b in range(B)]
            kt_sb = [pers.tile([128, S], BF16, name=f"kt{b}") for b in range(B)]
            # V tiles: per k-tile layout [h0 64 | ones | h1 64 | ones] (130 cols)
            v_sb = [pers.tile([128, NKT * 130], BF16, name=f"v{b}") for b in range(B)]
            wq_sb = pers.tile([128, 8, DHC], BF16, name="wq_sb")
            wk_sb = pers.tile([128, 8, DHC], BF16, name="wk_sb")
            wv_sb = pers.tile([128, 8, DHC], BF16, name="wv_sb")
            wo_sb = pers.tile([128, 8, D], BF16, name="wo_sb")
            tri_sb = pers.tile([128, 128], BF16, name="tri_sb")
            ones_sb = pers.tile([65, 64], BF16, name="ones_sb")
            id_b = pers.tile([128, 128], BF16, name="id_b")

            # ---- A2A staging: input (per-rank shards) and Shared output ----
            # k=0..2: (both b, m=k), 8 shards x [128, 128]
            # k=3: (b=0, m=3), k=4: (b=1, m=3): 8 shards x [128, 64]
            im_c = [
                dram.tile([N_CORES * DHC, 128], BF16, name=f"im{k}")
                for k in range(3)
            ] + [
                dram.tile([N_CORES * DHC, 64], BF16, name=f"im3{s}") for s in "ab"
            ]
            om_c = [
                dram.tile([N_CORES * DHC, 128], BF16, name=f"om{k}")
                for k in range(3)
            ] + [
                dram.tile([N_CORES * DHC, 64], BF16, name=f"om3{s}") for s in "ab"
            ]

            # warm-up collectives: the first two collectives of a NEFF pay a
            # large one-time setup (~40us + ~20us measured). Fire two tiny
            # AllToAlls at t=0 (no input deps -> no ring blocking) so the real
            # exchanges hit steady-state (~6us) cost.
            # id_b first: every x^T transpose depends on it, and gpsimd-ring
            # position IS execution order.
            make_identity(nc, id_b[:])
            nc.vector.memset(ones_sb[:], 1.0)
            nc.vector.memset(v_sb[0][:], 1.0)
            nc.vector.memset(v_sb[1][:], 1.0)
            dmy_i = dram.tile([8, 64], BF16, name="dmy_i")
            dmy_o = [dram.tile([8, 64], BF16, name=f"dmy_o{t}") for t in range(1)]
            for t_ in range(1):
                nc.gpsimd.collective_compute(
                    "AllToAll",
                    mybir.AluOpType.bypass,
                    replica_groups=[list(range(N_CORES))],
                    ins=[dmy_i[:]],
                    outs=[dmy_o[t_][:]],
                )

            # ---- gpsimd (SWDGE) ring: x cast-DMAs + weights; collectives later.
            # First 5 chunks upfront (pool bufs=5); chunks 5..7 are emitted later
            # (after the readers of the buffer they reuse), keeping all x DMAs
            # ahead of every AllToAll trigger-wait on this ring.
            x_bufs = {}
            x_fbufs = {}

            def x_dma(b, j):
                g = b * 4 + j
                rows = slice(g * 512, (g + 1) * 512)
                x_f = xfp.tile([128, 4, D], FP32, name="x_f", tag="xf")
                nc.sync.dma_start(
                    x_f[:], x_d[rows, :].rearrange("(c p) d -> p c d", p=128)
                )
                x_fbufs[(b, j)] = x_f

            x_dma(0, 0)
            for w_d, w_sb in ((wq_d, wq_sb), (wk_d, wk_sb), (wv_d, wv_sb)):
                nc.gpsimd.dma_start(
                    w_sb[:], w_d.rearrange("(c p) n -> p c n", p=128)
                )
            nc.gpsimd.dma_start(tri_sb[:], tri_d[:])
            x_dma(1, 0)
            nc.gpsimd.dma_start(
                wo_sb[:], wo_d.rearrange("(c p) n -> p c n", p=128)
            )

            # ---- filler queue ----
            Q = deque()

            def push(fn, tag=None):
                Q.append((tag, fn))

            def fill(n):
                for _ in range(min(n, len(Q))):
                    Q.popleft()[1]()

            def drain_until(tag):
                while Q:
                    t, fn = Q.popleft()
                    fn()
                    if t == tag:
                        break

            def drain_all():
                while Q:
                    Q.popleft()[1]()

            # toggle for eviction engine (spread PSUM-evict copies over
            # Vector and Scalar so neither becomes the bottleneck)
            _tgl = [0]

            def evict(dst, src):
                _tgl[0] ^= 1
                if _tgl[0]:
                    nc.vector.tensor_copy(dst, src)
                else:
                    nc.scalar.copy(dst, src)

            # ---- per-chunk prep: x^T (PE transpose) + QKV projections ----
            _proj_ps = {}
            _vt_sb = {}

            def push_prep(b, j):
                g = b * 4 + j
                cols = slice(b * S + j * QM, b * S + (j + 1) * QM)

                def cast():
                    x_f = x_fbufs.pop((b, j))
                    x_b = xbp.tile([128, 4, D], BF16, name="x_b", tag="xb")
                    nc.vector.tensor_copy(x_b[:], x_f[:])
                    x_bufs[(b, j)] = x_b

                push(cast)

                def tp(dt0):
                    def f():
                        x_b = x_bufs[(b, j)]
                        pst = ps_m.tile([128, 1024], BF16, name="pst", tag="m")
                        for dd in range(2):
                            dt = dt0 + dd
                            for c in range(4):
                                nc.tensor.transpose(
                                    pst[
                                        :,
                                        dd * 512 + c * 128 : dd * 512 + (c + 1) * 128,
                                    ],
                                    x_b[:, c, dt * 128 : (dt + 1) * 128],
                                    id_b[:],
                                )
                        nc.vector.tensor_copy(
                            xt_sb[:, dt0 : dt0 + 2, g * 512 : (g + 1) * 512],
                            pst[:].rearrange("p (d c) -> p d c", d=2),
                        )

                    return f

                for dt0 in (0, 2, 4, 6):
                    push(tp(dt0))

                def proj(w_sb, t_sb, half):
                    def f():
                        key = (b, j, id(w_sb))
                        if half == 0:
                            ps = ps_m.tile([128, QM], FP32, name="ps_p", tag="m")
                            _proj_ps[key] = ps
                        else:
                            ps = _proj_ps.pop(key)
                        for dt in range(4 * half, 4 * half + 4):
                            nc.tensor.matmul(
                                ps[:],
                                w_sb[:, dt, :],
                                xt_sb[:, dt, cols],
                                start=(dt == 0),
                                stop=(dt == 7),
                            )
                        if half == 1:
                            if t_sb is not None:
                                evict(t_sb[:, j * QM : (j + 1) * QM], ps[:])
                            else:
                                vt = nw.tile(
                                    [128, QM], BF16, name="vt", tag="vt", bufs=2
                                )
                                nc.vector.tensor_copy(vt[:], ps[:])
                                _vt_sb[(b, j)] = vt

                    return f

                push(proj(wq_sb, qt_sb[b], 0))
                push(proj(wq_sb, qt_sb[b], 1))
                push(proj(wk_sb, kt_sb[b], 0))
                push(proj(wk_sb, kt_sb[b], 1))
                push(proj(wv_sb, None, 0))
                push(proj(wv_sb, None, 1))

                def vtp():
                    vt = _vt_sb.pop((b, j))
                    pst = ps_m.tile([128, 1024], BF16, name="pst", tag="m")
                    for st in range(4):
                        nc.tensor.transpose(
                            pst[:, st * 128 : (st + 1) * 128],
                            vt[:, st * 128 : (st + 1) * 128],
                            id_b[:],
                        )
                    dst = v_sb[b][:, j * 520 : (j + 1) * 520].rearrange(
                        "p (t g c) -> p t g c", t=4, g=2
                    )[:, :, :, 0:64]
                    vsrc = pst[:, 0:512].rearrange("p (t g c) -> p t g c", t=4, g=2)
                    nc.vector.tensor_copy(dst, vsrc)

                push(vtp, tag=f"prep{b}{j}")

            # ---- attention for (b, m): scores + softmax + A@V; returns tail ----
            def attention(b, m):
                qcols = slice(m * QM, (m + 1) * QM)
                ctx_ps = [
                    ps_c.tile([65, QM], FP32, name=f"ctx_ps{h}", tag="c")
                    for h in range(HPC)
                ]
                n_kt = 4 * m + 4

                def score_mm(kt):
                    s_ps = ps_s.tile([128, 2 * QM], FP32, name="s_ps", tag="s")
                    for h in range(HPC):
                        nc.tensor.matmul(
                            s_ps[:, h * QM : (h + 1) * QM],
                            kt_sb[b][h * 64 : (h + 1) * 64, kt * 128 : (kt + 1) * 128],
                            qt_sb[b][h * 64 : (h + 1) * 64, qcols],
                            start=True,
                            stop=True,
                            tile_position=(h * 64, 0),
                        )
                    return s_ps

                s_cur = score_mm(0)
                for kt in range(n_kt):
                    s_nxt = score_mm(kt + 1) if kt + 1 < n_kt else None
                    s_ps = s_cur
                    j = kt - 4 * m  # diagonal block index if >= 0
                    qs = max(0, 128 * j)
                    pt = ptp.tile([128, 2 * QM], BF16, name="pt")
                    if j < 0:
                        nc.scalar.activation(pt[:], s_ps[:], Exp, scale=SCALE)
                    else:
                        for h in range(HPC):
                            nc.scalar.activation(
                                pt[:, h * QM + qs : (h + 1) * QM],
                                s_ps[:, h * QM + qs : (h + 1) * QM],
                                Exp,
                                scale=SCALE,
                            )
                            nc.vector.tensor_mul(
                                pt[:, h * QM + qs : h * QM + qs + 128],
                                pt[:, h * QM + qs : h * QM + qs + 128],
                                tri_sb[:],
                            )
                    fill(2)
                    for h in range(HPC):
                        nc.tensor.matmul(
                            ctx_ps[h][:, qs:QM],
                            v_sb[b][:, kt * 130 + h * 65 : kt * 130 + (h + 1) * 65],
                            pt[:, h * QM + qs : (h + 1) * QM],
                            start=(kt == 0),
                            stop=(kt == n_kt - 1),
                        )
                    s_cur = s_nxt
                # evict accumulators to SBUF (frees PSUM) + reciprocals now;
                # the PE-side normalize tail is deferred.
                # evict + reciprocal on the SCALAR engine: its queue position is
                # right after this attention's own exps, so the tail chain that
                # gates the AllToAll trigger is not stuck behind the (deep)
                # vector-engine backlog.
                ctxa_l, denb_l = [], []
                for h in range(HPC):
                    ctxa = nw.tile([65, QM], FP32, name="ctxa", tag="ctxa", bufs=4)
                    nc.scalar.copy(ctxa[:], ctx_ps[h][:])
                    denb = nw.tile([65, QM], BF16, name="denb", tag="recip", bufs=4)
                    with nc.allow_low_precision(reason="softmax denom to bf16"):
                        nc.scalar.copy(denb[64:65, :], ctxa[64:65, :])
                    ctxa_l.append(ctxa)
                    denb_l.append(denb)

                def tail():
                    # broadcast the (un-inverted) denom to 64 rows on the PE,
                    # then reciprocal on 128 partitions (a [1,512] 1-partition
                    # reciprocal costs ~4us; [128,512] costs ~0.4us).
                    bc_ps = ps_m.tile([128, QM], FP32, name="bc_ps", tag="m")
                    for h in range(HPC):
                        nc.tensor.matmul(
                            bc_ps[64 * h : 64 * h + 64, :],
                            ones_sb[64:65, :],
                            denb_l[h][64:65, :],
                            start=True,
                            stop=True,
                        )
                    rec = nw.tile([64, HPC * QM], FP32, name="rec", tag="rec", bufs=2)
                    for h in range(HPC):
                        nc.vector.reciprocal(
                            rec[:, h * QM : (h + 1) * QM],
                            bc_ps[64 * h : 64 * h + 64, :],
                        )
                    ctxn = nw.tile([128, QM], BF16, name="ctxn", tag="ctxn")
                    for h in range(HPC):
                        nc.vector.tensor_mul(
                            ctxn[64 * h : 64 * h + 64, :],
                            ctxa_l[h][0:64, :],
                            rec[:, h * QM : (h + 1) * QM],
                        )
                    if m < 3:
                        dst = im_c[m].rearrange("(s r) c -> r s c", r=128)[
                            :, 4 * b : 4 * b + 4, :
                        ]
                        src = ctxn[:].rearrange("r (p c) -> r p c", p=4)
                    else:
                        dst = im_c[3 + b].rearrange("(s r) c -> r s c", r=128)
                        src = ctxn[:].rearrange("r (s c) -> r s c", s=8)
                    nc.sync.dma_start(dst, src)
                    if DBG and b == 0 and m == 0:
                        nc.gpsimd.dma_start(dbg_ctxn[:, :], ctxn[:])

                return tail

            def a2a(k):
                nc.gpsimd.collective_compute(
                    "AllToAll",
                    mybir.AluOpType.bypass,
                    replica_groups=[list(range(N_CORES))],
                    ins=[im_c[k][:]],
                    outs=[om_c[k][:]],
                )

            # ---- output projection for A2A chunk k ----
            _cf = {}
            _of = {}

            def push_op(k):
                C = 128 if k < 3 else 64
                off = 128 * k if k < 3 else 384 + 64 * (k - 3)

                def cfdma():
                    cf = cfp.tile([128, 8, C], BF16, name="cf", tag="cf")
                    nc.sync.dma_start(
                        cf[:], om_c[k].rearrange("(t p) c -> p t c", p=128)
                    )
                    _cf[k] = cf
                    if DBG and k == 0:
                        nc.gpsimd.dma_start(
                            dbg_cf.rearrange("p (t c) -> p t c", t=8), cf[:]
                        )

                def mm(half):
                    def f():
                        cf = _cf[k]
                        if half == 0:
                            of = nw.tile([C, D], FP32, name="of", tag="of", bufs=2)
                            _of[k] = of
                        ps = ps_m.tile([C, 512], FP32, name="ps_o", tag="m")
                        for dt in range(8):
                            nc.tensor.matmul(
                                ps[:],
                                cf[:, dt, :],
                                wo_sb[:, dt, half * 512 : (half + 1) * 512],
                                start=(dt == 0),
                                stop=(dt == 7),
                            )
                        evict(_of[k][:, half * 512 : (half + 1) * 512], ps[:])
                        if half == 1:
                            of = _of.pop(k)
                            nc.sync.dma_start(out_d[off : off + C, :], of[:])

                    return f

                push(cfdma)
                push(mm(0))
                push(mm(1))

            # ---- main pipeline ----
            # chunk order: (0,0),(1,0),(0,1),(1,1),(0,2),(1,2),(0,3),(1,3)
            chunks = [(b, m) for m in range(NQ) for b in range(B)]
            push_prep(0, 0)
            drain_all()
            x_dma(0, 1)  # cast(0,0) emitted -> its x_f buffer is reusable
            # outproj thunks are spliced well after their AllToAll fires so the
            # strict-FIFO PE stream never blocks on a cf load.
            op_defer = {5: [0], 6: [1], 7: [2, 3]}
            for i, (b, m) in enumerate(chunks):
                if i + 1 < len(chunks):
                    nb, nm = chunks[i + 1]
                    push_prep(nb, nm)
                for k_ in op_defer.get(i, ()):
                    push_op(k_)
                t = attention(b, m)
                fill(2)
                t()
                if b == 1 and m < 3:
                    a2a(m)
                    if DBG and m == 0:
                        nc.gpsimd.dma_start(dbg_im[:, :], im_c[0][:])
                        nc.gpsimd.dma_start(dbg_om[:, :], om_c[0][:])
                if b == 0 and m == 3:
                    a2a(3)
                if i + 1 < len(chunks):
                    drain_until(f"prep{nb}{nm}")
                if i + 3 < len(chunks):
                    x_dma(*chunks[i + 3])  # cast(i+1) emitted -> buffer reusable
            a2a(4)
            drain_all()
            push_op(4)
            drain_all()
            if DBG:
                nc.gpsimd.dma_start(
                    dbg_xt.rearrange("p (t c) -> p t c", t=8), xt_sb[:, :, 0:512]
                )
                nc.gpsimd.dma_start(dbg_qt[:, :], qt_sb[0][:, 0:512])
                nc.gpsimd.dma_start(dbg_kt[:, :], kt_sb[0][:, 0:512])
                nc.gpsimd.dma_start(dbg_v[:, :], v_sb[0][:, 0:520])

    nc.compile()
    return nc


def _build_nc():
    if "nc" not in _nc_cache:
        _nc_cache["nc"] = _build()
    return _nc_cache["nc"]


def kernel(x, W_q, W_k, W_v, W_o):
    x = np.ascontiguousarray(np.asarray(x, dtype=np.float32)).reshape(BS, D)
    # keep-mask for the diagonal 128x128 block of S^T[k, q]: keep k <= q
    tri = np.triu(np.ones((128, 128), dtype=np.float32))
    wo_full = np.ascontiguousarray(np.asarray(W_o, np.float32))
    in_maps = []
    for c in range(N_CORES):
        sl = slice(c * DHC, (c + 1) * DHC)
        in_maps.append(
            {
                "x": x,
                "wq": np.ascontiguousarray(np.asarray(W_q, np.float32)[:, sl]),
                "wk": np.ascontiguousarray(np.asarray(W_k, np.float32)[:, sl]),
                "wv": np.ascontiguousarray(np.asarray(W_v, np.float32)[:, sl]),
                "wo": wo_full,
                "tri": tri,
            }
        )
    nc = _build_nc()
    res = run_bass_kernel_spmd(nc, in_maps, core_ids=list(range(N_CORES)))
    out = np.empty((B, S, D), dtype=np.float32)
    for c in range(N_CORES):
        oc = res.results[c]["out"]  # [512, 1024]
        bb, p = c // 4, c % 4
        for m in range(3):
            out[bb, m * QM + p * 128 : m * QM + (p + 1) * 128, :] = oc[
                m * 128 : (m + 1) * 128
            ]
        out[0, 3 * QM + 64 * c : 3 * QM + 64 * c + 64, :] = oc[384:448]
        out[1, 3 * QM + 64 * c : 3 * QM + 64 * c + 64, :] = oc[448:512]
    return out

